# revision 29
# baseline (speedup 1.0000x reference)
import numpy as np
import ml_dtypes

# CNN-biLSTM-CRF forward NLL, data-parallel over batch across 8 NeuronCores.
# Device side (per core, 8 sentences): word-embedding gather from a
# device-resident 120MB table via indirect DMA, input projections for both
# LSTM directions (one [331,1024]^T x [331,2048] matmul, bias folded in via
# a ones-row), the 128-step recurrence for forward+reverse simultaneously
# in a transposed layout (gate dim on partitions, batch on the free axis,
# so per-timestep slicing is a free-dim slice and no transposes are
# needed), the emission projection, and the full CRF (masked gold-score
# gather through a DRAM spill + 127-step logsumexp forward scan). Only
# [8,2] floats (logZ per example, gold-emission total) come back per core.
# Host: the tiny char-CNN, weight reordering/bf16 cast (device-resident,
# content-hash cached), and the tag/transition part of the gold score.
# Steady-state per-call traffic is ~0.6MB up / 0.5KB down; the wall time is
# dominated by the axon tunnel round-trip (~60-90ms), with ~2-3ms of actual
# device execution.

B, S, LW = 64, 128, 20
CHAR_E, CHAR_C = 30, 30
WORD_E = 300
H, NCLS = 256, 25
F = WORD_E + CHAR_C  # 330
KF = F + 1  # 331: ones row folds the gate bias into the big matmul
NCORES = 8
BC = B // NCORES  # 8 examples per core
R = BC * S  # 1024 batch*time columns per core
NW = 8 * H  # 2048 gate rows (both directions)
NQ = NW // 128  # 16 gate-dim chunks

BF16 = ml_dtypes.bfloat16


def _build_nc():
    import concourse.bacc as bacc
    import concourse.bass as bass
    import concourse.mybir as mybir
    from concourse import tile
    from concourse.masks import make_identity

    f32 = mybir.dt.float32
    bf16 = mybir.dt.bfloat16
    i32 = mybir.dt.int32
    SIG = mybir.ActivationFunctionType.Sigmoid
    TANH = mybir.ActivationFunctionType.Tanh
    EXP = mybir.ActivationFunctionType.Exp
    LN = mybir.ActivationFunctionType.Ln

    nc = bacc.Bacc("TRN2", target_bir_lowering=False, debug=False,
                   num_devices=NCORES)
    # resident across calls (content-hash cached on the host side):
    wtab = nc.dram_tensor("wtab", [100000, WORD_E], bf16,
                          kind="ExternalInput")
    wT = nc.dram_tensor("wT", [KF, NW], bf16, kind="ExternalInput")
    whhT = nc.dram_tensor("whhT", [H, NW], bf16, kind="ExternalInput")
    linwT = nc.dram_tensor("linwT", [2 * H, 64], bf16, kind="ExternalInput")
    # CRF constants [8, 700]: trans.T flat (625) | start (25) | end (25)
    # | lin_b (25), replicated across the batch partition dim
    crfc = nc.dram_tensor("crfc", [BC, 700], f32, kind="ExternalInput")
    # per-call: token ids [pos, chunk], charCNN features (+ones row),
    # mask [b, t], flat gold-tag gather indices [pos, chunk]
    sentx = nc.dram_tensor("sentx", [128, R // 128], i32,
                           kind="ExternalInput")
    cfT = nc.dram_tensor("cfT", [CHAR_C + 1, R], bf16, kind="ExternalInput")
    mkin = nc.dram_tensor("mk", [BC, S], i32, kind="ExternalInput")
    tgi = nc.dram_tensor("tgi", [128, R // 128], i32, kind="ExternalInput")
    # device-local spill of emissions for the gold-tag gather
    # flat index = b*(S*NCLS+1) + t*NCLS + cls; col S*NCLS is a zero cell
    EMW = S * NCLS + 1
    em_scr = nc.dram_tensor("emscr", [BC * EMW, 1], f32, kind="Internal")
    res_out = nc.dram_tensor("res", [BC, 2], f32, kind="ExternalOutput")

    KCH = [(0, 128), (128, 128), (256, KF - 256)]  # K chunks of big matmul
    # featT chunk 2 rows: word dims 256:300, char 0:30, ones
    DCH = [(0, 128), (128, 128), (256, 44)]  # word-emb dim chunks

    with tile.TileContext(nc) as tc:
        with tc.tile_pool(name="gx", bufs=1) as gxp:
            # gxT[p, q*R + t*BC + b]: gate-chunk q (128 gate dims on the
            # partition axis), time t, example b. fp32.
            gxT = gxp.tile([128, NQ * R], f32, tag="gxT")
            # ---- phase 0+1: gather word rows, transpose into featT
            # chunks, then gxT = ([w_ih;b]^T [feat;1]) ----
            with tc.tile_pool(name="lhs", bufs=1) as lp, \
                 tc.tile_pool(name="rhs", bufs=1) as rp, \
                 tc.tile_pool(name="wg", bufs=3) as wgp, \
                 tc.tile_pool(name="tp", bufs=2, space="PSUM") as tpp, \
                 tc.tile_pool(name="ps1", bufs=4, space="PSUM") as pp:
                ident = lp.tile([128, 128], bf16, tag="ident")
                make_identity(nc, ident[:, :])
                sx_sb = lp.tile([128, R // 128], i32, tag="sx")
                nc.sync.dma_start(sx_sb[:, :], sentx[:, :])
                lhs, rhs = [], []
                for k, (ko, kn) in enumerate(KCH):
                    lt = lp.tile([kn, NW], bf16, tag=f"l{k}")
                    nc.sync.dma_start(lt[:, :], wT[ko:ko + kn, :])
                    lhs.append(lt)
                    rt = rp.tile([kn, R], bf16, tag=f"r{k}")
                    rhs.append(rt)
                nc.sync.dma_start(rhs[2][44:KF - 256, :], cfT[:, :])
                for m in range(R // 128):
                    wg = wgp.tile([128, WORD_E], bf16, tag="wg")
                    nc.gpsimd.indirect_dma_start(
                        out=wg[:, :], out_offset=None, in_=wtab[:, :],
                        in_offset=bass.IndirectOffsetOnAxis(
                            ap=sx_sb[:, m:m + 1], axis=0))
                    for dk, (do, dn) in enumerate(DCH):
                        tp = tpp.tile([128, 128], bf16, tag="tp")
                        nc.tensor.transpose(tp[0:dn, :], wg[:, do:do + dn],
                                            ident[:, :])
                        nc.vector.tensor_copy(
                            rhs[dk][0:dn, m * 128:(m + 1) * 128],
                            tp[0:dn, :])
                for q in range(NQ):
                    for n in range(R // 512):
                        ps = pp.tile([128, 512], f32)
                        for k in range(3):
                            nc.tensor.matmul(
                                ps[:, :],
                                lhs[k][:, q * 128:(q + 1) * 128],
                                rhs[k][:, n * 512:(n + 1) * 512],
                                start=(k == 0), stop=(k == 2))
                        nc.vector.tensor_copy(
                            gxT[:, q * R + n * 512:q * R + (n + 1) * 512],
                            ps[:, :])

            # ---- phase 2: biLSTM recurrence, both directions per step ----
            # gate-chunk order q: [i_f,i_f, i_r,i_r, f_f,f_f, f_r,f_r,
            #                      o_f,o_f, o_r,o_r, g_f,g_f, g_r,g_r]
            # step tiles [128, 128]: free = q*BC + b. i=[0:32], f=[32:64],
            # o=[64:96], g~=[96:128]. h/c [128, 32]: fwd cols 0:16, rev 16:32.
            with tc.tile_pool(name="cst", bufs=1) as cp, \
                 tc.tile_pool(name="st", bufs=1) as st, \
                 tc.tile_pool(name="wk", bufs=2) as wk, \
                 tc.tile_pool(name="grp", bufs=2, space="PSUM") as grp, \
                 tc.tile_pool(name="emp", bufs=2, space="PSUM") as empp:
                whh_sb = []
                for k in range(2):
                    wt_ = cp.tile([128, NW], bf16, tag=f"whh{k}")
                    nc.sync.dma_start(wt_[:, :], whhT[k * 128:(k + 1) * 128, :])
                    whh_sb.append(wt_)
                linw_sb = []
                for k in range(4):
                    lw = cp.tile([128, 64], bf16, tag=f"lw{k}")
                    nc.sync.dma_start(lw[:, :], linwT[k * 128:(k + 1) * 128, :])
                    linw_sb.append(lw)

                c_t = st.tile([128, 32], f32, tag="c")
                h_t = st.tile([128, 32], bf16, tag="h")
                em_b = st.tile([BC, S * NCLS], f32, tag="emb")

                # [p, chunk q, v=t*BC+b] view of gxT for per-step slicing
                gxq = gxT[:, :].rearrange("p (q v) -> p q v", q=NQ, v=R)

                for tau in range(S):
                    tf, tr = tau, S - 1 - tau  # fwd/rev time this step
                    gates = wk.tile([128, 128], f32, tag="gates")
                    if tau == 0:
                        for d4 in range(8):  # 8 dir-groups of 2 chunks
                            t_ = tf if d4 % 2 == 0 else tr
                            nc.vector.tensor_copy(
                                gates[:, d4 * 16:(d4 + 1) * 16],
                                gxq[:, 2 * d4:2 * d4 + 2,
                                    t_ * BC:(t_ + 1) * BC])
                    else:
                        grs = grp.tile([128, 128], f32, tag="grs")
                        for q in range(NQ):
                            d = (q // 2) % 2  # 0 fwd, 1 rev
                            G = q // 4
                            j = q % 2
                            col = 1024 * d + 256 * G + 128 * j
                            for k in range(2):
                                nc.tensor.matmul(
                                    grs[:, q * BC:(q + 1) * BC],
                                    whh_sb[k][:, col:col + 128],
                                    h_t[:, 16 * d + BC * k:
                                        16 * d + BC * (k + 1)],
                                    start=(k == 0), stop=(k == 1))
                        for d4 in range(8):
                            t_ = tf if d4 % 2 == 0 else tr
                            nc.vector.tensor_add(
                                gates[:, d4 * 16:(d4 + 1) * 16],
                                gxq[:, 2 * d4:2 * d4 + 2,
                                    t_ * BC:(t_ + 1) * BC],
                                grs[:, d4 * 16:(d4 + 1) * 16])
                    sg = wk.tile([128, 128], f32, tag="sg")
                    nc.scalar.activation(sg[:, 0:96], gates[:, 0:96], SIG)
                    nc.scalar.activation(sg[:, 96:128], gates[:, 96:128],
                                         TANH)
                    if tau == 0:
                        nc.vector.tensor_mul(c_t[:, :], sg[:, 0:32],
                                             sg[:, 96:128])
                    else:
                        t1 = wk.tile([128, 32], f32, tag="t1")
                        t2 = wk.tile([128, 32], f32, tag="t2")
                        nc.vector.tensor_mul(t1[:, :], sg[:, 32:64], c_t[:, :])
                        nc.vector.tensor_mul(t2[:, :], sg[:, 0:32],
                                             sg[:, 96:128])
                        nc.vector.tensor_add(c_t[:, :], t1[:, :], t2[:, :])
                    th = wk.tile([128, 32], f32, tag="th")
                    nc.scalar.activation(th[:, :], c_t[:, :], TANH)
                    nc.vector.tensor_mul(h_t[:, :], sg[:, 64:96], th[:, :])

                    # emissions (batch-major): fwd h(tf) -> slot tf,
                    # rev h(tr) -> slot tr
                    emps = empp.tile([BC, 64], f32, tag="emps")
                    for k in range(2):
                        nc.tensor.matmul(
                            emps[:, 0:NCLS], h_t[:, BC * k:BC * (k + 1)],
                            linw_sb[k][:, 0:NCLS],
                            start=(k == 0), stop=(k == 1))
                    for k in range(2):
                        nc.tensor.matmul(
                            emps[:, 32:32 + NCLS],
                            h_t[:, 16 + BC * k:16 + BC * (k + 1)],
                            linw_sb[2 + k][:, 32:32 + NCLS],
                            start=(k == 0), stop=(k == 1))
                    if tau < S // 2:
                        nc.vector.tensor_copy(
                            em_b[:, tf * NCLS:(tf + 1) * NCLS],
                            emps[:, 0:NCLS])
                        nc.vector.tensor_copy(
                            em_b[:, tr * NCLS:(tr + 1) * NCLS],
                            emps[:, 32:32 + NCLS])
                    else:
                        nc.vector.tensor_add(
                            em_b[:, tf * NCLS:(tf + 1) * NCLS],
                            em_b[:, tf * NCLS:(tf + 1) * NCLS],
                            emps[:, 0:NCLS])
                        nc.vector.tensor_add(
                            em_b[:, tr * NCLS:(tr + 1) * NCLS],
                            em_b[:, tr * NCLS:(tr + 1) * NCLS],
                            emps[:, 32:32 + NCLS])

                # ---- phase 3: CRF on device ----
                crf_sb = cp.tile([BC, 700], f32, tag="crfc")
                nc.sync.dma_start(crf_sb[:, :], crfc[:, :])
                mk_sb = cp.tile([BC, S], i32, tag="mk")
                nc.sync.dma_start(mk_sb[:, :], mkin[:, :])
                tg_sb = cp.tile([128, R // 128], i32, tag="tg")
                nc.sync.dma_start(tg_sb[:, :], tgi[:, :])
                transT = crf_sb[:, 0:625]
                startt = crf_sb[:, 625:650]
                endt = crf_sb[:, 650:675]
                linb = crf_sb[:, 675:700]

                # em += lin_b (broadcast over time)
                linb_bc = bass.AP(linb.tensor, linb.offset,
                                  [linb.ap[0], [0, S], [1, NCLS]])
                em3 = em_b[:, :].rearrange("p (t c) -> p t c", t=S, c=NCLS)
                nc.vector.tensor_add(em3, em3, linb_bc)

                # spill emissions (+ zero cell) for the gold-tag gather
                scr8 = em_scr[:, :].rearrange("(a w) c -> a (w c)", a=BC,
                                              w=EMW)
                nc.sync.dma_start(scr8[:, 0:S * NCLS], em_b[:, :])
                zt = st.tile([BC, 1], f32, tag="zt")
                nc.vector.memset(zt[:, :], 0.0)
                nc.sync.dma_start(scr8[:, S * NCLS:EMW], zt[:, :])

                # masked gold-emission gather and total
                eg = st.tile([128, R // 128], f32, tag="eg")
                for m in range(R // 128):
                    nc.gpsimd.indirect_dma_start(
                        out=eg[:, m:m + 1], out_offset=None,
                        in_=em_scr[:, :],
                        in_offset=bass.IndirectOffsetOnAxis(
                            ap=tg_sb[:, m:m + 1], axis=0))
                egs = st.tile([128, 1], f32, tag="egs")
                nc.vector.tensor_reduce(egs[:, :], eg[:, :],
                                        axis=mybir.AxisListType.X,
                                        op=mybir.AluOpType.add)
                egt = st.tile([1, 1], f32, tag="egt")
                nc.gpsimd.tensor_reduce(egt[:, :], egs[:, :],
                                        axis=mybir.AxisListType.C,
                                        op=mybir.AluOpType.add)

                # forward algorithm (logsumexp scan over time)
                alpha = st.tile([BC, NCLS], f32, tag="alpha")
                nc.vector.tensor_add(alpha[:, :], startt, em_b[:, 0:NCLS])
                aap = alpha[:, :]
                alpha_bc = bass.AP(aap.tensor, aap.offset,
                                   [aap.ap[0], [0, NCLS], [1, NCLS]])
                for t in range(1, S):
                    z = wk.tile([BC, NCLS * NCLS], f32, tag="z")
                    z3 = z[:, :].rearrange("p (j i) -> p j i", j=NCLS, i=NCLS)
                    nc.vector.tensor_add(z3, transT[:, :].rearrange(
                        "p (j i) -> p j i", j=NCLS, i=NCLS), alpha_bc)
                    mx = wk.tile([BC, NCLS], f32, tag="mx")
                    nc.vector.tensor_reduce(mx[:, :], z3,
                                            axis=mybir.AxisListType.X,
                                            op=mybir.AluOpType.max)
                    z2 = wk.tile([BC, NCLS * NCLS], f32, tag="z2")
                    z23 = z2[:, :].rearrange("p (j i) -> p j i", j=NCLS,
                                             i=NCLS)
                    nc.vector.tensor_sub(z23, z3,
                                         mx[:, :].to_broadcast(
                                             [BC, NCLS, NCLS]))
                    nc.scalar.activation(z2[:, :], z2[:, :], EXP)
                    sm = wk.tile([BC, NCLS], f32, tag="sm")
                    nc.vector.tensor_reduce(sm[:, :], z23,
                                            axis=mybir.AxisListType.X,
                                            op=mybir.AluOpType.add)
                    nxt = wk.tile([BC, NCLS], f32, tag="nxt")
                    nc.scalar.activation(nxt[:, :], sm[:, :], LN)
                    nc.vector.tensor_add(nxt[:, :], nxt[:, :], mx[:, :])
                    nc.vector.tensor_add(nxt[:, :], nxt[:, :],
                                         em_b[:, t * NCLS:(t + 1) * NCLS])
                    nc.vector.copy_predicated(
                        alpha[:, :],
                        mk_sb[:, t:t + 1].to_broadcast([BC, NCLS]),
                        nxt[:, :])
                # logZ = logsumexp(alpha + end)
                fz = st.tile([BC, NCLS], f32, tag="fz")
                nc.vector.tensor_add(fz[:, :], alpha[:, :], endt)
                mxf = st.tile([BC, 1], f32, tag="mxf")
                nc.vector.tensor_reduce(mxf[:, :], fz[:, :],
                                        axis=mybir.AxisListType.X,
                                        op=mybir.AluOpType.max)
                nc.vector.tensor_sub(fz[:, :], fz[:, :],
                                     mxf[:, :].to_broadcast([BC, NCLS]))
                nc.scalar.activation(fz[:, :], fz[:, :], EXP)
                sf = st.tile([BC, 1], f32, tag="sf")
                nc.vector.tensor_reduce(sf[:, :], fz[:, :],
                                        axis=mybir.AxisListType.X,
                                        op=mybir.AluOpType.add)
                nc.scalar.activation(sf[:, :], sf[:, :], LN)
                res_sb = st.tile([BC, 2], f32, tag="res")
                nc.vector.memset(res_sb[:, :], 0.0)
                nc.vector.tensor_add(res_sb[:, 0:1], sf[:, :], mxf[:, :])
                nc.vector.tensor_copy(res_sb[0:1, 1:2], egt[:, :])
                nc.sync.dma_start(res_out[:, :], res_sb[:, :])
    nc.compile()
    return nc


_NC_CACHE = {}
LAST_DEVICE_NS = [0]


def _get_exec():
    # Persistent jitted executable (same _bass_exec_p lowering that
    # run_bass_kernel_spmd uses under axon, but cached across calls so we
    # don't pay a fresh shard_map trace/compile + replicated weight upload
    # on every invocation).
    if "exec" in _NC_CACHE:
        return _NC_CACHE["exec"]
    import jax
    from jax.sharding import Mesh, PartitionSpec, NamedSharding
    from jax.experimental.shard_map import shard_map
    import concourse.mybir as mybir
    from concourse import bass2jax

    bass2jax.install_neuronx_cc_hook()
    nc = _build_nc()
    assert nc.dbg_addr is None
    partition_name = (nc.partition_id_tensor.name
                      if nc.partition_id_tensor else None)
    in_names, out_names, out_avals, zero_outs = [], [], [], []
    for alloc in nc.m.functions[0].allocations:
        if not isinstance(alloc, mybir.MemoryLocationSet):
            continue
        name = alloc.memorylocations[0].name
        if alloc.kind == "ExternalInput":
            if name != partition_name:
                in_names.append(name)
        elif alloc.kind == "ExternalOutput":
            shape = tuple(alloc.tensor_shape)
            dtype = mybir.dt.np(alloc.dtype)
            out_names.append(name)
            out_avals.append(jax.core.ShapedArray(shape, dtype))
            zero_outs.append(np.zeros(shape, dtype))
    n_params = len(in_names)
    all_in_names = tuple(in_names + out_names
                         + ([partition_name] if partition_name else []))

    def _body(*args):
        operands = list(args)
        if partition_name is not None:
            operands.append(bass2jax.partition_id_tensor())
        outs = bass2jax._bass_exec_p.bind(
            *operands, out_avals=tuple(out_avals), in_names=all_in_names,
            out_names=tuple(out_names), lowering_input_output_aliases=(),
            sim_require_finite=True, sim_require_nnan=True, nc=nc)
        return tuple(outs)

    devices = jax.devices()[:NCORES]
    mesh = Mesh(np.asarray(devices), ("core",))
    nin = n_params + len(out_names)
    sharded = jax.jit(
        shard_map(_body, mesh=mesh,
                  in_specs=(PartitionSpec("core"),) * nin,
                  out_specs=(PartitionSpec("core"),) * len(out_names),
                  check_rep=False),
        keep_unused=True)
    sh = NamedSharding(mesh, PartitionSpec("core"))
    # output "pre-zero" operands: our kernel writes every output element,
    # so these are ballast — keep them resident instead of re-uploading
    zeros_dev = [jax.device_put(
        np.zeros((NCORES * z.shape[0], *z.shape[1:]), z.dtype), sh)
        for z in zero_outs]
    state = dict(sharded=sharded, sh=sh, devices=devices,
                 in_names=in_names, zeros=zeros_dev, wcache={})
    _NC_CACHE["exec"] = state
    return state


def _fingerprint(arr):
    # cheap content fingerprint: full md5 is too slow for the 120MB table
    import hashlib
    a = np.ascontiguousarray(arr)
    flat = a.reshape(-1)
    sample = np.ascontiguousarray(flat[::97])
    return (a.shape, str(a.dtype), hashlib.md5(sample.tobytes()).hexdigest(),
            float(np.float64(flat.view(np.uint8).reshape(-1)[:: 1013]
                             .astype(np.uint64).sum())))


_W_CAST = {"wtab": True}  # convert to bf16 lazily, only on cache miss


def _replicate(name, w, st):
    # replicated-per-core global array without materializing 8x on host
    import jax
    if _W_CAST.get(name) and w.dtype != BF16:
        w = w.astype(BF16)
    shards = [jax.device_put(w, d) for d in st["devices"]]
    return jax.make_array_from_single_device_arrays(
        (NCORES * w.shape[0],) + w.shape[1:], st["sh"], shards)


def _run_device(per_call, weights):
    # per_call: dict name -> list of per-core np arrays (shipped every call)
    # weights: dict name -> np array, same on every core (device-resident)
    import time
    st = _get_exec()
    key = tuple((n,) + _fingerprint(w) for n, w in sorted(weights.items()))
    if st["wcache"].get("key") != key:
        reps = {n: _replicate(n, w, st) for n, w in weights.items()}
        for r in reps.values():
            r.block_until_ready()
        st["wcache"] = {"key": key, "reps": reps}
    reps = st["wcache"]["reps"]
    args = []
    for name in st["in_names"]:
        if name in per_call:
            args.append(np.concatenate(per_call[name], axis=0))
        else:
            args.append(reps[name])
    import os
    t0 = time.time()
    outs = st["sharded"](*args, *st["zeros"])
    t1 = time.time()
    res_np = np.asarray(outs[0])
    t2 = time.time()
    LAST_DEVICE_NS[0] = int((t2 - t0) * 1e9)
    if os.environ.get("BK_TIMING"):
        print(f"[bk] dispatch+exec: {(t1 - t0) * 1e3:.1f} ms  "
              f"fetch: {(t2 - t1) * 1e3:.1f} ms")
    return res_np.reshape(NCORES, BC, 2)


def _gate_reord(w):
    # rows [i,f,g,o] (torch LSTM order) -> [i,f,o,g]
    return np.concatenate([w[0:H], w[H:2 * H], w[3 * H:4 * H], w[2 * H:3 * H]],
                          axis=0)


def kernel(word_table, char_table, conv_w, conv_b, w_ih_f, w_hh_f, b_f,
           w_ih_r, w_hh_r, b_r, lin_w, lin_b, start_t, end_t, trans,
           sent, word, tag, mask):
    word_table = np.asarray(word_table, np.float32)
    char_table = np.asarray(char_table, np.float32)
    conv_w = np.asarray(conv_w, np.float32)
    conv_b = np.asarray(conv_b, np.float32)
    lin_w = np.asarray(lin_w, np.float32)
    lin_b = np.asarray(lin_b, np.float32)
    start_t = np.asarray(start_t, np.float32)
    end_t = np.asarray(end_t, np.float32)
    trans = np.asarray(trans, np.float32)
    sent_i = np.asarray(sent).astype(np.int64)
    word_i = np.asarray(word).astype(np.int64)
    tag_i = np.asarray(tag).astype(np.int64)
    mask_b = np.asarray(mask).astype(bool)

    # --- char CNN (host) ---
    ct = char_table.copy()
    ct[0] = 0.0
    cemb = ct[word_i.reshape(-1)]  # [B*S*LW, CHAR_E]
    y1 = (cemb @ conv_w[:, :, 1].T).reshape(B * S, LW, CHAR_C)
    y0 = (cemb @ conv_w[:, :, 0].T).reshape(B * S, LW, CHAR_C)
    y2 = (cemb @ conv_w[:, :, 2].T).reshape(B * S, LW, CHAR_C)
    y1[:, 1:] += y0[:, :-1]
    y1[:, :-1] += y2[:, 1:]
    y1 += conv_b[None, None, :]
    char_feat = y1.max(axis=1).reshape(B, S, CHAR_C)

    # --- device weight prep (bf16) ---
    def gs(w):  # [4H, ...] -> i, f, g, o blocks
        return w[0:H], w[H:2 * H], w[2 * H:3 * H], w[3 * H:4 * H]

    i_f, f_f, g_f, o_f = gs(np.asarray(w_ih_f, np.float32))
    i_r, f_r, g_r, o_r = gs(np.asarray(w_ih_r, np.float32))
    wcat = np.concatenate([i_f, i_r, f_f, f_r, o_f, o_r, g_f, g_r], axis=0)
    bi_f, bf_f, bg_f, bo_f = gs(np.asarray(b_f, np.float32)[:, None])
    bi_r, bf_r, bg_r, bo_r = gs(np.asarray(b_r, np.float32)[:, None])
    bcat = np.concatenate([bi_f, bi_r, bf_f, bf_r, bo_f, bo_r, bg_f, bg_r],
                          axis=0)[:, 0]
    wT = np.zeros((KF, NW), np.float32)
    wT[:F] = wcat.T
    wT[F] = bcat
    wT = wT.astype(BF16)

    whhT = np.zeros((H, NW), np.float32)
    whhT[:, 0:4 * H] = _gate_reord(np.asarray(w_hh_f, np.float32)).T
    whhT[:, 4 * H:] = _gate_reord(np.asarray(w_hh_r, np.float32)).T
    whhT = whhT.astype(BF16)

    linwT = np.zeros((2 * H, 64), np.float32)
    linwT[0:H, 0:NCLS] = lin_w[:, 0:H].T
    linwT[H:2 * H, 32:32 + NCLS] = lin_w[:, H:2 * H].T
    linwT = linwT.astype(BF16)

    crfc = np.concatenate([trans.T.reshape(-1), start_t, end_t, lin_b])
    crfc = np.tile(crfc[None, :], (BC, 1)).astype(np.float32)  # [8, 700]

    EMW = S * NCLS + 1
    mask_f = mask_b.astype(np.float32)  # [B, S]
    sentx_shards, cfT_shards, mk_shards, tgi_shards = [], [], [], []
    for c in range(NCORES):
        sl = slice(c * BC, (c + 1) * BC)
        # token order r = t*BC + b; sentx[pos, chunk] with r = chunk*128+pos
        rs = sent_i[sl].T.reshape(R)
        sentx_shards.append(
            np.ascontiguousarray(rs.reshape(R // 128, 128).T)
            .astype(np.int32))
        cf = np.ones((CHAR_C + 1, R), np.float32)
        cf[:CHAR_C] = char_feat[sl].transpose(2, 1, 0).reshape(CHAR_C, R)
        cfT_shards.append(cf.astype(BF16))
        mk_shards.append(mask_b[sl].astype(np.int32))
        # flat gold-tag indices into em_scr; masked slots hit the zero cell
        bl = np.arange(BC)[:, None]
        idx = bl * EMW + np.arange(S)[None, :] * NCLS + tag_i[sl]
        idx = np.where(mask_b[sl], idx, bl * EMW + S * NCLS)  # [BC, S]
        tgi_shards.append(
            np.ascontiguousarray(
                idx.reshape(R).reshape(R // 128, 128).T).astype(np.int32))

    res = _run_device(
        {"sentx": sentx_shards, "cfT": cfT_shards, "mk": mk_shards,
         "tgi": tgi_shards},
        {"wtab": word_table, "wT": wT, "whhT": whhT, "linwT": linwT,
         "crfc": crfc})  # [NCORES, BC, 2]

    logZ_sum = float(np.float64(res[:, :, 0].sum()))
    em_tag_sum = float(np.float64(res[:, 0, 1].sum()))

    # tag/transition part of the gold score (host: no em needed)
    tg = tag_i.T  # [S,B]
    mk = mask_f.T
    bidx = np.arange(B)
    tr = trans[tg[:-1], tg[1:]]
    tag_part = start_t[tg[0]] + np.sum(mk[1:] * tr, axis=0)
    last = mk.sum(0).astype(np.int64) - 1
    tag_part = tag_part + end_t[tg[last, bidx]]
    nll = logZ_sum - (em_tag_sum + float(np.float64(tag_part.sum())))
    return np.asarray(nll, np.float32)


# revision 34
# speedup vs baseline: 1.3033x; 1.3033x over previous
import numpy as np
import ml_dtypes

# CNN-biLSTM-CRF forward NLL, data-parallel over batch across 8 NeuronCores.
# Device side (per core, 8 sentences): word-embedding gather from a
# device-resident 120MB table via indirect DMA, input projections for both
# LSTM directions (one [331,1024]^T x [331,2048] matmul, bias folded in via
# a ones-row), the 128-step recurrence for forward+reverse simultaneously
# in a transposed layout (gate dim on partitions, batch on the free axis,
# so per-timestep slicing is a free-dim slice and no transposes are
# needed), the emission projection, and the full CRF (masked gold-score
# gather through a DRAM spill + 127-step logsumexp forward scan). Only
# [8,2] floats (logZ per example, gold-emission total) come back per core.
# Host: the tiny char-CNN, weight reordering/bf16 cast (device-resident,
# content-hash cached), and the tag/transition part of the gold score.
# Steady-state per-call traffic is ~0.6MB up / 0.5KB down; the wall time is
# dominated by the axon tunnel round-trip (~60-90ms), with ~2-3ms of actual
# device execution.

B, S, LW = 64, 128, 20
CHAR_E, CHAR_C = 30, 30
WORD_E = 300
H, NCLS = 256, 25
F = WORD_E + CHAR_C  # 330
KF = F + 1  # 331: ones row folds the gate bias into the big matmul
NCORES = 8
BC = B // NCORES  # 8 examples per core
R = BC * S  # 1024 batch*time columns per core
NW = 8 * H  # 2048 gate rows (both directions)
NQ = NW // 128  # 16 gate-dim chunks

BF16 = ml_dtypes.bfloat16


def _build_nc():
    import concourse.bacc as bacc
    import concourse.bass as bass
    import concourse.mybir as mybir
    from concourse import tile
    from concourse.masks import make_identity

    f32 = mybir.dt.float32
    bf16 = mybir.dt.bfloat16
    i32 = mybir.dt.int32
    SIG = mybir.ActivationFunctionType.Sigmoid
    TANH = mybir.ActivationFunctionType.Tanh
    EXP = mybir.ActivationFunctionType.Exp
    LN = mybir.ActivationFunctionType.Ln

    nc = bacc.Bacc("TRN2", target_bir_lowering=False, debug=False,
                   num_devices=NCORES)
    # resident across calls (content-hash cached on the host side):
    wtab = nc.dram_tensor("wtab", [100000, WORD_E], bf16,
                          kind="ExternalInput")
    wT = nc.dram_tensor("wT", [KF, NW], bf16, kind="ExternalInput")
    whhT = nc.dram_tensor("whhT", [H, NW], bf16, kind="ExternalInput")
    linwT = nc.dram_tensor("linwT", [2 * H, 64], bf16, kind="ExternalInput")
    # CRF constants [8, 700]: (unused 625) | start (25) | exp(end) (25)
    # | lin_b (25), replicated across the batch partition dim
    crfc = nc.dram_tensor("crfc", [BC, 700], f32, kind="ExternalInput")
    # exp(trans) [from i, to j] for the linear-domain forward scan
    texp = nc.dram_tensor("texp", [NCLS, NCLS], f32, kind="ExternalInput")
    # per-call: token ids [pos, chunk], charCNN features (+ones row),
    # mask [b, t], flat gold-tag gather indices [pos, chunk]
    sentx = nc.dram_tensor("sentx", [128, R // 128], i32,
                           kind="ExternalInput")
    cfT = nc.dram_tensor("cfT", [CHAR_C + 1, R], bf16, kind="ExternalInput")
    mkin = nc.dram_tensor("mk", [BC, S], i32, kind="ExternalInput")
    tgi = nc.dram_tensor("tgi", [128, R // 128], i32, kind="ExternalInput")
    # device-local spill of emissions for the gold-tag gather
    # flat index = b*(S*NCLS+1) + t*NCLS + cls; col S*NCLS is a zero cell
    EMW = S * NCLS + 1
    em_scr = nc.dram_tensor("emscr", [BC * EMW, 1], f32, kind="Internal")
    res_out = nc.dram_tensor("res", [BC, 2], f32, kind="ExternalOutput")

    KCH = [(0, 128), (128, 128), (256, KF - 256)]  # K chunks of big matmul
    # featT chunk 2 rows: word dims 256:300, char 0:30, ones
    DCH = [(0, 128), (128, 128), (256, 44)]  # word-emb dim chunks

    with tile.TileContext(nc) as tc:
        with tc.tile_pool(name="gx", bufs=1) as gxp:
            # gxT[p, q*R + t*BC + b]: gate-chunk q (128 gate dims on the
            # partition axis), time t, example b. fp32.
            gxT = gxp.tile([128, NQ * R], f32, tag="gxT")
            # ---- phase 0+1: gather word rows, transpose into featT
            # chunks, then gxT = ([w_ih;b]^T [feat;1]) ----
            with tc.tile_pool(name="lhs", bufs=1) as lp, \
                 tc.tile_pool(name="rhs", bufs=1) as rp, \
                 tc.tile_pool(name="wg", bufs=3) as wgp, \
                 tc.tile_pool(name="tp", bufs=2, space="PSUM") as tpp, \
                 tc.tile_pool(name="ps1", bufs=4, space="PSUM") as pp:
                ident = lp.tile([128, 128], bf16, tag="ident")
                make_identity(nc, ident[:, :])
                sx_sb = lp.tile([128, R // 128], i32, tag="sx")
                nc.sync.dma_start(sx_sb[:, :], sentx[:, :])
                lhs, rhs = [], []
                for k, (ko, kn) in enumerate(KCH):
                    lt = lp.tile([kn, NW], bf16, tag=f"l{k}")
                    nc.sync.dma_start(lt[:, :], wT[ko:ko + kn, :])
                    lhs.append(lt)
                    rt = rp.tile([kn, R], bf16, tag=f"r{k}")
                    rhs.append(rt)
                nc.sync.dma_start(rhs[2][44:KF - 256, :], cfT[:, :])
                for m in range(R // 128):
                    wg = wgp.tile([128, WORD_E], bf16, tag="wg")
                    nc.gpsimd.indirect_dma_start(
                        out=wg[:, :], out_offset=None, in_=wtab[:, :],
                        in_offset=bass.IndirectOffsetOnAxis(
                            ap=sx_sb[:, m:m + 1], axis=0))
                    for dk, (do, dn) in enumerate(DCH):
                        tp = tpp.tile([128, 128], bf16, tag="tp")
                        nc.tensor.transpose(tp[0:dn, :], wg[:, do:do + dn],
                                            ident[:, :])
                        nc.vector.tensor_copy(
                            rhs[dk][0:dn, m * 128:(m + 1) * 128],
                            tp[0:dn, :])
                for q in range(NQ):
                    for n in range(R // 512):
                        ps = pp.tile([128, 512], f32)
                        for k in range(3):
                            nc.tensor.matmul(
                                ps[:, :],
                                lhs[k][:, q * 128:(q + 1) * 128],
                                rhs[k][:, n * 512:(n + 1) * 512],
                                start=(k == 0), stop=(k == 2))
                        nc.vector.tensor_copy(
                            gxT[:, q * R + n * 512:q * R + (n + 1) * 512],
                            ps[:, :])

            # ---- phase 2: biLSTM recurrence, both directions per step ----
            # gate-chunk order q: [i_f,i_f, i_r,i_r, f_f,f_f, f_r,f_r,
            #                      o_f,o_f, o_r,o_r, g_f,g_f, g_r,g_r]
            # step tiles [128, 128]: free = q*BC + b. i=[0:32], f=[32:64],
            # o=[64:96], g~=[96:128]. h/c [128, 32]: fwd cols 0:16, rev 16:32.
            with tc.tile_pool(name="cst", bufs=1) as cp, \
                 tc.tile_pool(name="st", bufs=1) as st, \
                 tc.tile_pool(name="wk", bufs=2) as wk, \
                 tc.tile_pool(name="grp", bufs=2, space="PSUM") as grp, \
                 tc.tile_pool(name="emp", bufs=2, space="PSUM") as empp:
                whh_sb = []
                for k in range(2):
                    wt_ = cp.tile([128, NW], bf16, tag=f"whh{k}")
                    nc.sync.dma_start(wt_[:, :], whhT[k * 128:(k + 1) * 128, :])
                    whh_sb.append(wt_)
                linw_sb = []
                for k in range(4):
                    lw = cp.tile([128, 64], bf16, tag=f"lw{k}")
                    nc.sync.dma_start(lw[:, :], linwT[k * 128:(k + 1) * 128, :])
                    linw_sb.append(lw)

                c_t = st.tile([128, 32], f32, tag="c")
                h_t = st.tile([128, 32], bf16, tag="h")
                em_b = st.tile([BC, S * NCLS], f32, tag="emb")

                # [p, chunk q, v=t*BC+b] view of gxT for per-step slicing
                gxq = gxT[:, :].rearrange("p (q v) -> p q v", q=NQ, v=R)

                for tau in range(S):
                    tf, tr = tau, S - 1 - tau  # fwd/rev time this step
                    gates = wk.tile([128, 128], f32, tag="gates")
                    if tau == 0:
                        for d4 in range(8):  # 8 dir-groups of 2 chunks
                            t_ = tf if d4 % 2 == 0 else tr
                            nc.vector.tensor_copy(
                                gates[:, d4 * 16:(d4 + 1) * 16],
                                gxq[:, 2 * d4:2 * d4 + 2,
                                    t_ * BC:(t_ + 1) * BC])
                    else:
                        grs = grp.tile([128, 128], f32, tag="grs")
                        for q in range(NQ):
                            d = (q // 2) % 2  # 0 fwd, 1 rev
                            G = q // 4
                            j = q % 2
                            col = 1024 * d + 256 * G + 128 * j
                            for k in range(2):
                                nc.tensor.matmul(
                                    grs[:, q * BC:(q + 1) * BC],
                                    whh_sb[k][:, col:col + 128],
                                    h_t[:, 16 * d + BC * k:
                                        16 * d + BC * (k + 1)],
                                    start=(k == 0), stop=(k == 1))
                        for d4 in range(8):
                            t_ = tf if d4 % 2 == 0 else tr
                            nc.vector.tensor_add(
                                gates[:, d4 * 16:(d4 + 1) * 16],
                                gxq[:, 2 * d4:2 * d4 + 2,
                                    t_ * BC:(t_ + 1) * BC],
                                grs[:, d4 * 16:(d4 + 1) * 16])
                    sg = wk.tile([128, 128], f32, tag="sg")
                    nc.scalar.activation(sg[:, 0:96], gates[:, 0:96], SIG)
                    nc.scalar.activation(sg[:, 96:128], gates[:, 96:128],
                                         TANH)
                    if tau == 0:
                        nc.vector.tensor_mul(c_t[:, :], sg[:, 0:32],
                                             sg[:, 96:128])
                    else:
                        t1 = wk.tile([128, 32], f32, tag="t1")
                        t2 = wk.tile([128, 32], f32, tag="t2")
                        nc.vector.tensor_mul(t1[:, :], sg[:, 32:64], c_t[:, :])
                        nc.vector.tensor_mul(t2[:, :], sg[:, 0:32],
                                             sg[:, 96:128])
                        nc.vector.tensor_add(c_t[:, :], t1[:, :], t2[:, :])
                    th = wk.tile([128, 32], f32, tag="th")
                    nc.scalar.activation(th[:, :], c_t[:, :], TANH)
                    nc.vector.tensor_mul(h_t[:, :], sg[:, 64:96], th[:, :])

                    # emissions (batch-major): fwd h(tf) -> slot tf,
                    # rev h(tr) -> slot tr
                    emps = empp.tile([BC, 64], f32, tag="emps")
                    for k in range(2):
                        nc.tensor.matmul(
                            emps[:, 0:NCLS], h_t[:, BC * k:BC * (k + 1)],
                            linw_sb[k][:, 0:NCLS],
                            start=(k == 0), stop=(k == 1))
                    for k in range(2):
                        nc.tensor.matmul(
                            emps[:, 32:32 + NCLS],
                            h_t[:, 16 + BC * k:16 + BC * (k + 1)],
                            linw_sb[2 + k][:, 32:32 + NCLS],
                            start=(k == 0), stop=(k == 1))
                    if tau < S // 2:
                        nc.vector.tensor_copy(
                            em_b[:, tf * NCLS:(tf + 1) * NCLS],
                            emps[:, 0:NCLS])
                        nc.vector.tensor_copy(
                            em_b[:, tr * NCLS:(tr + 1) * NCLS],
                            emps[:, 32:32 + NCLS])
                    else:
                        nc.vector.tensor_add(
                            em_b[:, tf * NCLS:(tf + 1) * NCLS],
                            em_b[:, tf * NCLS:(tf + 1) * NCLS],
                            emps[:, 0:NCLS])
                        nc.vector.tensor_add(
                            em_b[:, tr * NCLS:(tr + 1) * NCLS],
                            em_b[:, tr * NCLS:(tr + 1) * NCLS],
                            emps[:, 32:32 + NCLS])

                # ---- phase 3: CRF on device (linear-domain forward) ----
                crf_sb = cp.tile([BC, 700], f32, tag="crfc")
                nc.sync.dma_start(crf_sb[:, :], crfc[:, :])
                texp_sb = cp.tile([NCLS, NCLS], f32, tag="texp")
                nc.sync.dma_start(texp_sb[:, :], texp[:, :])
                idf = cp.tile([32, 32], f32, tag="idf")
                make_identity(nc, idf[:, :])
                mk_sb = cp.tile([BC, S], i32, tag="mk")
                nc.sync.dma_start(mk_sb[:, :], mkin[:, :])
                mkf_sb = cp.tile([BC, S], f32, tag="mkf")
                nc.vector.tensor_copy(mkf_sb[:, :], mk_sb[:, :])
                tg_sb = cp.tile([128, R // 128], i32, tag="tg")
                nc.sync.dma_start(tg_sb[:, :], tgi[:, :])
                startt = crf_sb[:, 625:650]
                eendt = crf_sb[:, 650:675]
                linb = crf_sb[:, 675:700]

                # em += lin_b (broadcast over time)
                linb_bc = bass.AP(linb.tensor, linb.offset,
                                  [linb.ap[0], [0, S], [1, NCLS]])
                em3 = em_b[:, :].rearrange("p (t c) -> p t c", t=S, c=NCLS)
                nc.vector.tensor_add(em3, em3, linb_bc)

                # spill emissions (+ zero cell) for the gold-tag gather
                scr8 = em_scr[:, :].rearrange("(a w) c -> a (w c)", a=BC,
                                              w=EMW)
                nc.sync.dma_start(scr8[:, 0:S * NCLS], em_b[:, :])
                zt = st.tile([BC, 1], f32, tag="zt")
                nc.vector.memset(zt[:, :], 0.0)
                nc.sync.dma_start(scr8[:, S * NCLS:EMW], zt[:, :])

                # masked gold-emission gather and total
                eg = st.tile([128, R // 128], f32, tag="eg")
                for m in range(R // 128):
                    nc.gpsimd.indirect_dma_start(
                        out=eg[:, m:m + 1], out_offset=None,
                        in_=em_scr[:, :],
                        in_offset=bass.IndirectOffsetOnAxis(
                            ap=tg_sb[:, m:m + 1], axis=0))
                egs = st.tile([128, 1], f32, tag="egs")
                nc.vector.tensor_reduce(egs[:, :], eg[:, :],
                                        axis=mybir.AxisListType.X,
                                        op=mybir.AluOpType.add)
                egt = st.tile([1, 1], f32, tag="egt")
                nc.gpsimd.tensor_reduce(egt[:, :], egs[:, :],
                                        axis=mybir.AxisListType.C,
                                        op=mybir.AluOpType.add)

                # per-timestep emission max + exp(em - mx): bulk, 3 ops
                em3b = em_b[:, :].rearrange("p (t c) -> p t c", t=S, c=NCLS)
                emx = st.tile([BC, S], f32, tag="emx")
                nc.vector.tensor_reduce(emx[:, :], em3b,
                                        axis=mybir.AxisListType.X,
                                        op=mybir.AluOpType.max)
                eap = emx[:, :]
                emx_bc = bass.AP(eap.tensor, eap.offset,
                                 [eap.ap[0], [1, S], [0, NCLS]])
                eme = st.tile([BC, S * NCLS], f32, tag="eme")
                eme3 = eme[:, :].rearrange("p (t c) -> p t c", t=S, c=NCLS)
                nc.vector.tensor_sub(eme3, em3b, emx_bc)
                nc.scalar.activation(eme[:, :], eme[:, :], EXP)

                # acc = mx0' + sum_{t>=1} mask_t * emx_t  (+ rescale logs)
                # where mx0' = max_j(start + em_0); A = exp-domain alpha
                a0 = st.tile([BC, NCLS], f32, tag="a0")
                nc.vector.tensor_add(a0[:, :], startt, em_b[:, 0:NCLS])
                mx0 = st.tile([BC, 1], f32, tag="mx0")
                nc.vector.tensor_reduce(mx0[:, :], a0[:, :],
                                        axis=mybir.AxisListType.X,
                                        op=mybir.AluOpType.max)
                A = st.tile([BC, NCLS], f32, tag="A")
                nc.vector.tensor_sub(A[:, :], a0[:, :],
                                     mx0[:, :].to_broadcast([BC, NCLS]))
                nc.scalar.activation(A[:, :], A[:, :], EXP)
                acc = st.tile([BC, 1], f32, tag="acc")
                mm_ = st.tile([BC, S], f32, tag="mm_")
                nc.vector.tensor_mul(mm_[:, 1:S], mkf_sb[:, 1:S],
                                     emx[:, 1:S])
                nc.vector.tensor_reduce(acc[:, :], mm_[:, 1:S],
                                        axis=mybir.AxisListType.X,
                                        op=mybir.AluOpType.add)
                nc.vector.tensor_add(acc[:, :], acc[:, :], mx0[:, :])

                for t in range(1, S):
                    # A' = (A @ exp(trans)) * exp(em_t - mx_t), masked
                    atp = grp.tile([NCLS, BC], f32, tag="atp")
                    nc.tensor.transpose(atp[:, :], A[:, :], idf[0:BC, 0:BC])
                    at_sb = wk.tile([NCLS, BC], f32, tag="at")
                    nc.vector.tensor_copy(at_sb[:, :], atp[:, :])
                    zps = grp.tile([BC, NCLS], f32, tag="zps")
                    nc.tensor.matmul(zps[:, :], at_sb[:, :], texp_sb[:, :],
                                     start=True, stop=True)
                    an = wk.tile([BC, NCLS], f32, tag="an")
                    nc.vector.tensor_mul(an[:, :], zps[:, :],
                                         eme[:, t * NCLS:(t + 1) * NCLS])
                    nc.vector.copy_predicated(
                        A[:, :],
                        mk_sb[:, t:t + 1].to_broadcast([BC, NCLS]),
                        an[:, :])
                    if t % 16 == 0:
                        # rescale to keep A in fp32 range
                        rm = wk.tile([BC, 1], f32, tag="rm")
                        nc.vector.tensor_reduce(rm[:, :], A[:, :],
                                                axis=mybir.AxisListType.X,
                                                op=mybir.AluOpType.max)
                        rr = wk.tile([BC, 1], f32, tag="rr")
                        nc.vector.reciprocal(rr[:, :], rm[:, :])
                        nc.vector.tensor_mul(A[:, :], A[:, :],
                                             rr[:, :].to_broadcast(
                                                 [BC, NCLS]))
                        nc.scalar.activation(rm[:, :], rm[:, :], LN)
                        nc.vector.tensor_add(acc[:, :], acc[:, :], rm[:, :])

                # logZ = acc + ln(sum_j A_j * exp(end_j))
                fz = st.tile([BC, NCLS], f32, tag="fz")
                nc.vector.tensor_mul(fz[:, :], A[:, :], eendt)
                sf = st.tile([BC, 1], f32, tag="sf")
                nc.vector.tensor_reduce(sf[:, :], fz[:, :],
                                        axis=mybir.AxisListType.X,
                                        op=mybir.AluOpType.add)
                nc.scalar.activation(sf[:, :], sf[:, :], LN)
                res_sb = st.tile([BC, 2], f32, tag="res")
                nc.vector.memset(res_sb[:, :], 0.0)
                nc.vector.tensor_add(res_sb[:, 0:1], sf[:, :], acc[:, :])
                nc.vector.tensor_copy(res_sb[0:1, 1:2], egt[:, :])
                nc.sync.dma_start(res_out[:, :], res_sb[:, :])
    nc.compile()
    return nc


_NC_CACHE = {}
LAST_DEVICE_NS = [0]


def _get_exec():
    # Persistent jitted executable (same _bass_exec_p lowering that
    # run_bass_kernel_spmd uses under axon, but cached across calls so we
    # don't pay a fresh shard_map trace/compile + replicated weight upload
    # on every invocation).
    if "exec" in _NC_CACHE:
        return _NC_CACHE["exec"]
    import jax
    from jax.sharding import Mesh, PartitionSpec, NamedSharding
    from jax.experimental.shard_map import shard_map
    import concourse.mybir as mybir
    from concourse import bass2jax

    bass2jax.install_neuronx_cc_hook()
    nc = _build_nc()
    assert nc.dbg_addr is None
    partition_name = (nc.partition_id_tensor.name
                      if nc.partition_id_tensor else None)
    in_names, out_names, out_avals, zero_outs = [], [], [], []
    for alloc in nc.m.functions[0].allocations:
        if not isinstance(alloc, mybir.MemoryLocationSet):
            continue
        name = alloc.memorylocations[0].name
        if alloc.kind == "ExternalInput":
            if name != partition_name:
                in_names.append(name)
        elif alloc.kind == "ExternalOutput":
            shape = tuple(alloc.tensor_shape)
            dtype = mybir.dt.np(alloc.dtype)
            out_names.append(name)
            out_avals.append(jax.core.ShapedArray(shape, dtype))
            zero_outs.append(np.zeros(shape, dtype))
    n_params = len(in_names)
    all_in_names = tuple(in_names + out_names
                         + ([partition_name] if partition_name else []))

    def _body(*args):
        operands = list(args)
        if partition_name is not None:
            operands.append(bass2jax.partition_id_tensor())
        outs = bass2jax._bass_exec_p.bind(
            *operands, out_avals=tuple(out_avals), in_names=all_in_names,
            out_names=tuple(out_names), lowering_input_output_aliases=(),
            sim_require_finite=True, sim_require_nnan=True, nc=nc)
        return tuple(outs)

    devices = jax.devices()[:NCORES]
    mesh = Mesh(np.asarray(devices), ("core",))
    nin = n_params + len(out_names)
    sharded = jax.jit(
        shard_map(_body, mesh=mesh,
                  in_specs=(PartitionSpec("core"),) * nin,
                  out_specs=(PartitionSpec("core"),) * len(out_names),
                  check_rep=False),
        keep_unused=True)
    sh = NamedSharding(mesh, PartitionSpec("core"))
    # output "pre-zero" operands: our kernel writes every output element,
    # so these are ballast — keep them resident instead of re-uploading
    zeros_dev = [jax.device_put(
        np.zeros((NCORES * z.shape[0], *z.shape[1:]), z.dtype), sh)
        for z in zero_outs]
    state = dict(sharded=sharded, sh=sh, devices=devices,
                 in_names=in_names, zeros=zeros_dev, wcache={})
    _NC_CACHE["exec"] = state
    return state


def _fingerprint(arr):
    # cheap content fingerprint: full md5 is too slow for the 120MB table
    import hashlib
    a = np.ascontiguousarray(arr)
    flat = a.reshape(-1)
    sample = np.ascontiguousarray(flat[::97])
    return (a.shape, str(a.dtype), hashlib.md5(sample.tobytes()).hexdigest(),
            float(np.float64(flat.view(np.uint8).reshape(-1)[:: 1013]
                             .astype(np.uint64).sum())))


_W_CAST = {"wtab": True}  # convert to bf16 lazily, only on cache miss


def _replicate(name, w, st):
    # replicated-per-core global array without materializing 8x on host
    import jax
    if _W_CAST.get(name) and w.dtype != BF16:
        w = w.astype(BF16)
    shards = [jax.device_put(w, d) for d in st["devices"]]
    return jax.make_array_from_single_device_arrays(
        (NCORES * w.shape[0],) + w.shape[1:], st["sh"], shards)


def _run_device(per_call, weights):
    # per_call: dict name -> list of per-core np arrays (shipped every call)
    # weights: dict name -> np array, same on every core (device-resident)
    import time
    st = _get_exec()
    key = tuple((n,) + _fingerprint(w) for n, w in sorted(weights.items()))
    if st["wcache"].get("key") != key:
        reps = {n: _replicate(n, w, st) for n, w in weights.items()}
        for r in reps.values():
            r.block_until_ready()
        st["wcache"] = {"key": key, "reps": reps}
    reps = st["wcache"]["reps"]
    args = []
    for name in st["in_names"]:
        if name in per_call:
            args.append(np.concatenate(per_call[name], axis=0))
        else:
            args.append(reps[name])
    import os
    t0 = time.time()
    outs = st["sharded"](*args, *st["zeros"])
    t1 = time.time()
    res_np = np.asarray(outs[0])
    t2 = time.time()
    LAST_DEVICE_NS[0] = int((t2 - t0) * 1e9)
    if os.environ.get("BK_TIMING"):
        print(f"[bk] dispatch+exec: {(t1 - t0) * 1e3:.1f} ms  "
              f"fetch: {(t2 - t1) * 1e3:.1f} ms")
    return res_np.reshape(NCORES, BC, 2)


def _gate_reord(w):
    # rows [i,f,g,o] (torch LSTM order) -> [i,f,o,g]
    return np.concatenate([w[0:H], w[H:2 * H], w[3 * H:4 * H], w[2 * H:3 * H]],
                          axis=0)


def kernel(word_table, char_table, conv_w, conv_b, w_ih_f, w_hh_f, b_f,
           w_ih_r, w_hh_r, b_r, lin_w, lin_b, start_t, end_t, trans,
           sent, word, tag, mask):
    word_table = np.asarray(word_table, np.float32)
    char_table = np.asarray(char_table, np.float32)
    conv_w = np.asarray(conv_w, np.float32)
    conv_b = np.asarray(conv_b, np.float32)
    lin_w = np.asarray(lin_w, np.float32)
    lin_b = np.asarray(lin_b, np.float32)
    start_t = np.asarray(start_t, np.float32)
    end_t = np.asarray(end_t, np.float32)
    trans = np.asarray(trans, np.float32)
    sent_i = np.asarray(sent).astype(np.int64)
    word_i = np.asarray(word).astype(np.int64)
    tag_i = np.asarray(tag).astype(np.int64)
    mask_b = np.asarray(mask).astype(bool)

    # --- char CNN (host) ---
    ct = char_table.copy()
    ct[0] = 0.0
    cemb = ct[word_i.reshape(-1)]  # [B*S*LW, CHAR_E]
    y1 = (cemb @ conv_w[:, :, 1].T).reshape(B * S, LW, CHAR_C)
    y0 = (cemb @ conv_w[:, :, 0].T).reshape(B * S, LW, CHAR_C)
    y2 = (cemb @ conv_w[:, :, 2].T).reshape(B * S, LW, CHAR_C)
    y1[:, 1:] += y0[:, :-1]
    y1[:, :-1] += y2[:, 1:]
    y1 += conv_b[None, None, :]
    char_feat = y1.max(axis=1).reshape(B, S, CHAR_C)

    # --- device weight prep (bf16) ---
    def gs(w):  # [4H, ...] -> i, f, g, o blocks
        return w[0:H], w[H:2 * H], w[2 * H:3 * H], w[3 * H:4 * H]

    i_f, f_f, g_f, o_f = gs(np.asarray(w_ih_f, np.float32))
    i_r, f_r, g_r, o_r = gs(np.asarray(w_ih_r, np.float32))
    wcat = np.concatenate([i_f, i_r, f_f, f_r, o_f, o_r, g_f, g_r], axis=0)
    bi_f, bf_f, bg_f, bo_f = gs(np.asarray(b_f, np.float32)[:, None])
    bi_r, bf_r, bg_r, bo_r = gs(np.asarray(b_r, np.float32)[:, None])
    bcat = np.concatenate([bi_f, bi_r, bf_f, bf_r, bo_f, bo_r, bg_f, bg_r],
                          axis=0)[:, 0]
    wT = np.zeros((KF, NW), np.float32)
    wT[:F] = wcat.T
    wT[F] = bcat
    wT = wT.astype(BF16)

    whhT = np.zeros((H, NW), np.float32)
    whhT[:, 0:4 * H] = _gate_reord(np.asarray(w_hh_f, np.float32)).T
    whhT[:, 4 * H:] = _gate_reord(np.asarray(w_hh_r, np.float32)).T
    whhT = whhT.astype(BF16)

    linwT = np.zeros((2 * H, 64), np.float32)
    linwT[0:H, 0:NCLS] = lin_w[:, 0:H].T
    linwT[H:2 * H, 32:32 + NCLS] = lin_w[:, H:2 * H].T
    linwT = linwT.astype(BF16)

    crfc = np.concatenate([trans.T.reshape(-1), start_t, np.exp(end_t),
                           lin_b])
    crfc = np.tile(crfc[None, :], (BC, 1)).astype(np.float32)  # [8, 700]
    texp = np.exp(trans).astype(np.float32)  # [25, 25]

    EMW = S * NCLS + 1
    mask_f = mask_b.astype(np.float32)  # [B, S]
    sentx_shards, cfT_shards, mk_shards, tgi_shards = [], [], [], []
    for c in range(NCORES):
        sl = slice(c * BC, (c + 1) * BC)
        # token order r = t*BC + b; sentx[pos, chunk] with r = chunk*128+pos
        rs = sent_i[sl].T.reshape(R)
        sentx_shards.append(
            np.ascontiguousarray(rs.reshape(R // 128, 128).T)
            .astype(np.int32))
        cf = np.ones((CHAR_C + 1, R), np.float32)
        cf[:CHAR_C] = char_feat[sl].transpose(2, 1, 0).reshape(CHAR_C, R)
        cfT_shards.append(cf.astype(BF16))
        mk_shards.append(mask_b[sl].astype(np.int32))
        # flat gold-tag indices into em_scr; masked slots hit the zero cell
        bl = np.arange(BC)[:, None]
        idx = bl * EMW + np.arange(S)[None, :] * NCLS + tag_i[sl]
        idx = np.where(mask_b[sl], idx, bl * EMW + S * NCLS)  # [BC, S]
        tgi_shards.append(
            np.ascontiguousarray(
                idx.reshape(R).reshape(R // 128, 128).T).astype(np.int32))

    res = _run_device(
        {"sentx": sentx_shards, "cfT": cfT_shards, "mk": mk_shards,
         "tgi": tgi_shards},
        {"wtab": word_table, "wT": wT, "whhT": whhT, "linwT": linwT,
         "crfc": crfc, "texp": texp})  # [NCORES, BC, 2]

    logZ_sum = float(np.float64(res[:, :, 0].sum()))
    em_tag_sum = float(np.float64(res[:, 0, 1].sum()))

    # tag/transition part of the gold score (host: no em needed)
    tg = tag_i.T  # [S,B]
    mk = mask_f.T
    bidx = np.arange(B)
    tr = trans[tg[:-1], tg[1:]]
    tag_part = start_t[tg[0]] + np.sum(mk[1:] * tr, axis=0)
    last = mk.sum(0).astype(np.int64) - 1
    tag_part = tag_part + end_t[tg[last, bidx]]
    nll = logZ_sum - (em_tag_sum + float(np.float64(tag_part.sum())))
    return np.asarray(nll, np.float32)
